# revision 10
# baseline (speedup 1.0000x reference)
"""Trainium2 Bass kernel for nn_CrossAttention (B=4, C=256, H=W=64, RC=32).

Sharding: 8 cores = (batch b in 0..3) x (query-pixel half in 0..1).
Each core gets x[b][:, nloc] (256 x 2048), the full context[b] (256 x 4096),
and replicated weights; it computes out[b][:, nloc] (256 x 2048).
No collectives: k/v are recomputed on both cores sharing a batch item
(cheap relative to the attention matmuls).

Per-core math (all on-chip, fp32 with float32r matmuls):
  q  (32,  nloc) = WqT.T @ x      (+bq)
  k  (32,  4096) = WkT.T @ ctx    (+bk)
  vT (4096, 257) = ctx.T @ WvT    (col 256 = ones -> free row-sum)
  per 512-col strip of n:
    for each of 32 m-tiles: logitsT (128m, 512n) = k_tile.T @ q_strip  (K=32)
      attnT = exp(logitsT * 1/sqrt(32))               [ScalarE]
      av[j] (128n, 257) += attnT[:, j*128:...].T @ vT_tile  (K=128)
    per 128-row n-tile j: normalize by col 256 (row-sum), scale by gamma,
      PE-transpose to (d, n), add x + gamma*bv, DMA out.
"""

import math

import numpy as np

import concourse.bass as bass
import concourse.mybir as mybir
import concourse.tile as tile
from concourse import bacc, masks
from concourse.bass_utils import run_bass_kernel_spmd

B, C, RC = 4, 256, 32
H = W = 64
NPIX = H * W          # 4096 query pixels per batch item
M = NPIX              # context pixels
NLOC = NPIX // 2      # query pixels per core
P = 128
N_CORES = 8
NSTRIPS = NLOC // 512  # 4
MT = M // P            # 32 m-tiles
F32 = mybir.dt.float32
FR = mybir.dt.float32r
SCALE = 1.0 / math.sqrt(RC)

_CACHE = {}


def _r(ap):
    """View an fp32 AP as float32r for single-pass PE matmuls."""
    return ap.bitcast(FR)


def _bcast_part(ap, p):
    """Partition-broadcast a DRAM AP of shape (k,) to (p, k) via step-0 AP."""
    return bass.AP(tensor=ap.tensor, offset=ap.offset, ap=[[0, p]] + list(ap.ap))


def build_nc():
    nc = bacc.Bacc("TRN2", target_bir_lowering=False, debug=False)
    xs = nc.dram_tensor("xs", [C, NLOC], F32, kind="ExternalInput").ap()
    cfd = nc.dram_tensor("cf", [C, M], F32, kind="ExternalInput").ap()
    wqT = nc.dram_tensor("WqT", [C, RC], F32, kind="ExternalInput").ap()
    bq = nc.dram_tensor("bq", [RC], F32, kind="ExternalInput").ap()
    wkT = nc.dram_tensor("WkT", [C, RC], F32, kind="ExternalInput").ap()
    bk = nc.dram_tensor("bk", [RC], F32, kind="ExternalInput").ap()
    wvT = nc.dram_tensor("WvT", [C, C], F32, kind="ExternalInput").ap()
    bv = nc.dram_tensor("bv", [C], F32, kind="ExternalInput").ap()
    gamma = nc.dram_tensor("gamma", [1], F32, kind="ExternalInput").ap()
    o_dram = nc.dram_tensor("out", [C, NLOC], F32, kind="ExternalOutput").ap()

    with tile.TileContext(nc) as tc:
        _emit(tc, xs, cfd, wqT, bq, wkT, bk, wvT, bv, gamma, o_dram)
    nc.compile()
    return nc


def _emit(tc, xs, cfd, wqT_d, bq, wkT_d, bk, wvT_d, bv, gamma, o_dram):
    nc = tc.nc
    from contextlib import ExitStack

    with ExitStack() as ctx:
        const = ctx.enter_context(tc.tile_pool(name="const", bufs=1))

        ident = const.tile([P, P], F32)
        masks.make_identity(nc, ident)

        # ---- load weights / biases (pre-transposed on host) -------------
        wqT = const.tile([P, 2, RC], FR)    # [c_in_chunk, ci, r]
        wkT = const.tile([P, 2, RC], FR)
        wvT = const.tile([P, 2, C], FR)     # [c_in_chunk, ci, d]
        for ci in range(2):
            csl = slice(ci * P, (ci + 1) * P)
            nc.sync.dma_start(out=wqT[:, ci, :], in_=_r(wqT_d[csl, :]))
            nc.sync.dma_start(out=wkT[:, ci, :], in_=_r(wkT_d[csl, :]))
            nc.sync.dma_start(out=wvT[:, ci, :], in_=_r(wvT_d[csl, :]))
        bq_sb = const.tile([RC, 1], F32)
        nc.sync.dma_start(out=bq_sb, in_=bq.unsqueeze(1))
        bk_sb = const.tile([RC, 1], F32)
        nc.sync.dma_start(out=bk_sb, in_=bk.unsqueeze(1))
        bv_sb = const.tile([P, 2, 1], F32)  # [p, dj, 1]
        for dj in range(2):
            nc.sync.dma_start(
                out=bv_sb[:, dj, :], in_=bv[dj * P:(dj + 1) * P].unsqueeze(1)
            )
        gamma_bc = const.tile([P, 1], F32)
        nc.sync.dma_start(out=gamma_bc, in_=_bcast_part(gamma, P))

        # ---- load x and context -----------------------------------------
        xf = const.tile([P, 2, NLOC], FR)   # [p, ci, n] rounded, feeds q matmul
        xb = const.tile([P, 2, NLOC], F32)  # exact copy for the residual
        for ci in range(2):
            nc.sync.dma_start(out=xf[:, ci, :], in_=_r(xs[ci * P:(ci + 1) * P, :]))
            nc.sync.dma_start(out=xb[:, ci, :], in_=xs[ci * P:(ci + 1) * P, :])
        cf = const.tile([P, 2, M], FR)      # [p, ci, m]
        for ci in range(2):
            for pc in range(4):
                sl = slice(pc * 1024, (pc + 1) * 1024)
                nc.sync.dma_start(
                    out=cf[:, ci, sl], in_=_r(cfd[ci * P:(ci + 1) * P, sl])
                )

        # ---- projections -------------------------------------------------
        q_sb = const.tile([RC, NLOC], FR)
        k_sb = const.tile([RC, M], FR)
        # col 256 = 1.0 (free row-sum), col 257 = 0.0 (fp32r needs even N)
        vT = const.tile([P, MT, 264], FR)  # [m_in_tile, mt, d]
        for mt in range(MT):
            nc.gpsimd.memset(vT[:, mt, 256:258].bitcast(F32), 0.0)
            nc.gpsimd.memset(vT[:, mt, 256:257].bitcast(F32), 1.0)

        with tc.tile_pool(name="psQ", bufs=2, space="PSUM") as psQ:
            for s in range(NSTRIPS):
                sl = slice(s * 512, (s + 1) * 512)
                pq = psQ.tile([RC, 512], F32, tag="pq")
                for ci in range(2):
                    nc.tensor.matmul(
                        pq, wqT[:, ci, :], xf[:, ci, sl],
                        start=(ci == 0), stop=(ci == 1),
                    )
                nc.vector.tensor_scalar_add(q_sb[:, sl], pq, bq_sb)
            for s in range(M // 512):
                sl = slice(s * 512, (s + 1) * 512)
                pk = psQ.tile([RC, 512], F32, tag="pq")
                for ci in range(2):
                    nc.tensor.matmul(
                        pk, wkT[:, ci, :], cf[:, ci, sl],
                        start=(ci == 0), stop=(ci == 1),
                    )
                nc.vector.tensor_scalar_add(k_sb[:, sl], pk, bk_sb)

        with tc.tile_pool(name="psV", bufs=2, space="PSUM") as psV:
            for mt in range(MT):
                msl = slice(mt * P, (mt + 1) * P)
                pv = psV.tile([P, C], F32, tag="pvv")
                for ci in range(2):
                    nc.tensor.matmul(
                        pv, cf[:, ci, msl], wvT[:, ci, :],
                        start=(ci == 0), stop=(ci == 1),
                    )
                nc.vector.tensor_copy(vT[:, mt, 0:256], pv)

        # xb = x + gamma*bv (per-partition), written in place over xf
        gvb = const.tile([P, 2, 1], F32)
        for dj in range(2):
            nc.vector.tensor_scalar_mul(gvb[:, dj, :], bv_sb[:, dj, :], gamma_bc)
            nc.vector.tensor_scalar_add(xb[:, dj, :], xb[:, dj, :], gvb[:, dj, :])

        # ---- attention ---------------------------------------------------
        with ExitStack() as bctx:
            psL = bctx.enter_context(tc.tile_pool(name="psL", bufs=2, space="PSUM"))
            psAV = bctx.enter_context(tc.tile_pool(name="psAV", bufs=4, space="PSUM"))
            psT = bctx.enter_context(tc.tile_pool(name="psT", bufs=2, space="PSUM"))
            attn = bctx.enter_context(tc.tile_pool(name="attn", bufs=6))
            eps = bctx.enter_context(tc.tile_pool(name="eps", bufs=4))

            for s in range(NSTRIPS):
                nsl = slice(s * 512, (s + 1) * 512)
                av = [psAV.tile([P, 512], F32, tag="av", name="av") for _ in range(4)]
                for mt in range(MT):
                    msl = slice(mt * P, (mt + 1) * P)
                    pl = psL.tile([P, 512], F32, tag="pl")
                    nc.tensor.matmul(
                        pl, k_sb[:, msl], q_sb[:, nsl],
                        start=True, stop=True,
                    )
                    at = attn.tile([P, 512], FR, tag="at")
                    nc.scalar.activation(
                        out=at, in_=pl,
                        func=mybir.ActivationFunctionType.Exp, scale=SCALE,
                    )
                    for j in range(4):
                        nc.tensor.matmul(
                            av[j][:, 0:258],
                            at[:, j * P:(j + 1) * P],
                            vT[:, mt, 0:258],
                            start=(mt == 0), stop=(mt == MT - 1),
                        )
                # strip epilogue
                pT = [psT.tile([P, 512], F32, tag="pT", name="pT") for _ in range(2)]
                for j in range(4):
                    rec = eps.tile([P, 1], F32, tag="rec")
                    nc.vector.reciprocal(rec, av[j][:, 256:257])
                    recg = eps.tile([P, 1], F32, tag="recg")
                    nc.vector.tensor_scalar_mul(recg, rec, gamma_bc)
                    o_sb = eps.tile([P, 256], F32, tag="o_sb")
                    nc.vector.tensor_scalar_mul(o_sb, av[j][:, 0:256], recg)
                    for dj in range(2):
                        nc.tensor.transpose(
                            pT[dj][:, j * P:(j + 1) * P],
                            o_sb[:, dj * P:(dj + 1) * P],
                            ident,
                        )
                for dj in range(2):
                    res = eps.tile([P, 512], F32, tag="res")
                    nc.vector.tensor_add(res, pT[dj], xb[:, dj, nsl])
                    nc.sync.dma_start(
                        out=o_dram[dj * P:(dj + 1) * P, nsl], in_=res
                    )


def _shard_inputs(x, context, Wq, bq, Wk, bk, Wv, bv, gamma):
    xb = np.ascontiguousarray(np.asarray(x, dtype=np.float32)).reshape(B, C, NPIX)
    cb = np.ascontiguousarray(np.asarray(context, dtype=np.float32)).reshape(B, C, NPIX)
    shared = {
        "WqT": np.ascontiguousarray(np.asarray(Wq, dtype=np.float32).T),
        "bq": np.ascontiguousarray(np.asarray(bq, dtype=np.float32)),
        "WkT": np.ascontiguousarray(np.asarray(Wk, dtype=np.float32).T),
        "bk": np.ascontiguousarray(np.asarray(bk, dtype=np.float32)),
        "WvT": np.ascontiguousarray(np.asarray(Wv, dtype=np.float32).T),
        "bv": np.ascontiguousarray(np.asarray(bv, dtype=np.float32)),
        "gamma": np.ascontiguousarray(np.asarray(gamma, dtype=np.float32)),
    }
    in_maps = []
    for core in range(N_CORES):
        b, half = core // 2, core % 2
        m = dict(shared)
        m["xs"] = np.ascontiguousarray(xb[b][:, half * NLOC:(half + 1) * NLOC])
        m["cf"] = np.ascontiguousarray(cb[b])
        in_maps.append(m)
    return in_maps


def _gather(results):
    out = np.empty((B, C, NPIX), dtype=np.float32)
    for core in range(N_CORES):
        b, half = core // 2, core % 2
        out[b][:, half * NLOC:(half + 1) * NLOC] = results[core]["out"]
    return out.reshape(B, C, H, W)


def run(inputs, trace=False, **kw):
    """Build (cached), run on the 8 NeuronCores, return (output, results)."""
    if "nc" not in _CACHE:
        _CACHE["nc"] = build_nc()
    nc = _CACHE["nc"]
    in_maps = _shard_inputs(**inputs)
    res = run_bass_kernel_spmd(
        nc, in_maps, core_ids=list(range(N_CORES)), trace=trace, **kw
    )
    return _gather(res.results), res


def kernel(**inputs) -> np.ndarray:
    out, _ = run(inputs, trace=False)
    return out


# revision 13
# speedup vs baseline: 385.3091x; 385.3091x over previous
"""Trainium2 Bass kernel for nn_CrossAttention (B=4, C=256, H=W=64, RC=32).

Sharding: 8 cores = (batch b in 0..3) x (query-pixel half in 0..1).
Each core gets x[b][:, nloc] (256 x 2048), the full context[b] (256 x 4096),
and replicated weights; it computes out[b][:, nloc] (256 x 2048).
No collectives: k/v are recomputed on both cores sharing a batch item
(cheap relative to the attention matmuls).

Per-core math (all on-chip, fp32 with float32r matmuls):
  q  (32,  nloc) = WqT.T @ x      (+bq)
  k  (32,  4096) = WkT.T @ ctx    (+bk)
  vT (4096, 257) = ctx.T @ WvT    (col 256 = ones -> free row-sum)
  per 512-col strip of n:
    for each of 32 m-tiles: logitsT (128m, 512n) = k_tile.T @ q_strip  (K=32)
      attnT = exp(logitsT * 1/sqrt(32))               [ScalarE]
      av[j] (128n, 257) += attnT[:, j*128:...].T @ vT_tile  (K=128)
    per 128-row n-tile j: normalize by col 256 (row-sum), scale by gamma,
      PE-transpose to (d, n), add x + gamma*bv, DMA out.
"""

import math

import numpy as np

import concourse.bass as bass
import concourse.mybir as mybir
import concourse.tile as tile
from concourse import bacc, masks
from concourse.bass_utils import run_bass_kernel_spmd

B, C, RC = 4, 256, 32
H = W = 64
NPIX = H * W          # 4096 query pixels per batch item
M = NPIX              # context pixels
NLOC = NPIX // 2      # query pixels per core
P = 128
N_CORES = 8
NSTRIPS = NLOC // 512  # 4
MT = M // P            # 32 m-tiles
F32 = mybir.dt.float32
FR = mybir.dt.float32r
SCALE = 1.0 / math.sqrt(RC)

_CACHE = {}


def _r(ap):
    """View an fp32 AP as float32r for single-pass PE matmuls."""
    return ap.bitcast(FR)


def _bcast_part(ap, p):
    """Partition-broadcast a DRAM AP of shape (k,) to (p, k) via step-0 AP."""
    return bass.AP(tensor=ap.tensor, offset=ap.offset, ap=[[0, p]] + list(ap.ap))


def build_nc(reps=1):
    nc = bacc.Bacc("TRN2", target_bir_lowering=False, debug=False)
    xs = nc.dram_tensor("xs", [C, NLOC], F32, kind="ExternalInput").ap()
    cfd = nc.dram_tensor("cf", [C, M], F32, kind="ExternalInput").ap()
    wqT = nc.dram_tensor("WqT", [C, RC], F32, kind="ExternalInput").ap()
    bq = nc.dram_tensor("bq", [RC], F32, kind="ExternalInput").ap()
    wkT = nc.dram_tensor("WkT", [C, RC], F32, kind="ExternalInput").ap()
    bk = nc.dram_tensor("bk", [RC], F32, kind="ExternalInput").ap()
    wvT = nc.dram_tensor("WvT", [C, C], F32, kind="ExternalInput").ap()
    bv = nc.dram_tensor("bv", [C], F32, kind="ExternalInput").ap()
    gamma = nc.dram_tensor("gamma", [1], F32, kind="ExternalInput").ap()
    o_dram = nc.dram_tensor("out", [C, NLOC], F32, kind="ExternalOutput").ap()

    with tile.TileContext(nc) as tc:
        for _ in range(reps):
            _emit(tc, xs, cfd, wqT, bq, wkT, bk, wvT, bv, gamma, o_dram)
    nc.compile()
    return nc


def _emit(tc, xs, cfd, wqT_d, bq, wkT_d, bk, wvT_d, bv, gamma, o_dram):
    nc = tc.nc
    from contextlib import ExitStack

    with ExitStack() as ctx:
        const = ctx.enter_context(tc.tile_pool(name="const", bufs=1))

        ident = const.tile([P, P], F32)
        masks.make_identity(nc, ident)

        # ---- loads, ordered by when PE needs them -----------------------
        # xf + wqT first (unblock q), then cf pieces + wkT/wvT (k, vT),
        # then biases; xb (residual copy) last. DMA issue is spread across
        # engine sequencers so SP doesn't serialize the prologue.
        xf = const.tile([P, 2, NLOC], FR)   # [p, ci, n] rounded, feeds q matmul
        for ci in range(2):
            nc.sync.dma_start(out=xf[:, ci, :], in_=_r(xs[ci * P:(ci + 1) * P, :]))
        wqT = const.tile([P, 2, RC], FR)    # [c_in_chunk, ci, r]
        wkT = const.tile([P, 2, RC], FR)
        wvT = const.tile([P, 2, C], FR)     # [c_in_chunk, ci, d]
        for ci in range(2):
            csl = slice(ci * P, (ci + 1) * P)
            nc.scalar.dma_start(out=wqT[:, ci, :], in_=_r(wqT_d[csl, :]))
            nc.scalar.dma_start(out=wkT[:, ci, :], in_=_r(wkT_d[csl, :]))
            nc.gpsimd.dma_start(out=wvT[:, ci, :], in_=_r(wvT_d[csl, :]))
        # context in 8 independently-landing pieces [ci][pc] of (128, 1024)
        cfp = [
            [
                const.tile([P, 1024], FR, name=f"cf_{ci}_{pc}", tag=f"cf_{ci}_{pc}")
                for pc in range(4)
            ]
            for ci in range(2)
        ]
        dma_engines = [nc.sync, nc.scalar, nc.gpsimd]
        di = 0
        for pc in range(4):
            for ci in range(2):
                eng = dma_engines[di % 3]
                di += 1
                eng.dma_start(
                    out=cfp[ci][pc],
                    in_=_r(cfd[ci * P:(ci + 1) * P, pc * 1024:(pc + 1) * 1024]),
                )
        bq_sb = const.tile([RC, 1], F32)
        nc.gpsimd.dma_start(out=bq_sb, in_=bq.unsqueeze(1))
        bk_sb = const.tile([RC, 1], F32)
        nc.gpsimd.dma_start(out=bk_sb, in_=bk.unsqueeze(1))
        bv_sb = const.tile([P, 2, 1], F32)  # [p, dj, 1]
        for dj in range(2):
            nc.gpsimd.dma_start(
                out=bv_sb[:, dj, :], in_=bv[dj * P:(dj + 1) * P].unsqueeze(1)
            )
        gamma_bc = const.tile([P, 1], F32)
        nc.gpsimd.dma_start(out=gamma_bc, in_=_bcast_part(gamma, P))
        xb = const.tile([P, 2, NLOC], F32)  # exact x copy for the residual
        for ci in range(2):
            nc.sync.dma_start(out=xb[:, ci, :], in_=xs[ci * P:(ci + 1) * P, :])

        # ---- projections -------------------------------------------------
        q_sb = const.tile([RC, NLOC], FR)
        k_sb = const.tile([RC, M], FR)
        # col 256 = 1.0 (free row-sum), col 257 = 0.0 (fp32r needs even N)
        vT = const.tile([P, MT, 264], FR)  # [m_in_tile, mt, d]
        for mt in range(MT):
            nc.gpsimd.memset(vT[:, mt, 256:258].bitcast(F32), 0.0)
            nc.gpsimd.memset(vT[:, mt, 256:257].bitcast(F32), 1.0)

        with tc.tile_pool(name="psQ", bufs=2, space="PSUM") as psQ, \
             tc.tile_pool(name="psV", bufs=2, space="PSUM") as psV:
            for sq in range(NSTRIPS):
                sl = slice(sq * 512, (sq + 1) * 512)
                pq = psQ.tile([RC, 512], F32, tag="pq")
                for ci in range(2):
                    nc.tensor.matmul(
                        pq, wqT[:, ci, :], xf[:, ci, sl],
                        start=(ci == 0), stop=(ci == 1),
                    )
                nc.vector.tensor_scalar_add(q_sb[:, sl], pq, bq_sb)
            # k and vT interleaved per cf piece, following DMA arrival order
            for pc in range(4):
                for sk in range(2):
                    sl = slice((pc * 2 + sk) * 512, (pc * 2 + sk + 1) * 512)
                    psl = slice(sk * 512, (sk + 1) * 512)
                    pk = psQ.tile([RC, 512], F32, tag="pq")
                    for ci in range(2):
                        nc.tensor.matmul(
                            pk, wkT[:, ci, :], cfp[ci][pc][:, psl],
                            start=(ci == 0), stop=(ci == 1),
                        )
                    nc.vector.tensor_scalar_add(k_sb[:, sl], pk, bk_sb)
                for mi in range(8):
                    mt = pc * 8 + mi
                    psl = slice(mi * P, (mi + 1) * P)
                    pv = psV.tile([P, C], F32, tag="pvv")
                    for ci in range(2):
                        nc.tensor.matmul(
                            pv, cfp[ci][pc][:, psl], wvT[:, ci, :],
                            start=(ci == 0), stop=(ci == 1),
                        )
                    nc.vector.tensor_copy(vT[:, mt, 0:256], pv)

        # xb = x + gamma*bv (per-partition), written in place over xf
        gvb = const.tile([P, 2, 1], F32)
        for dj in range(2):
            nc.vector.tensor_scalar_mul(gvb[:, dj, :], bv_sb[:, dj, :], gamma_bc)
            nc.vector.tensor_scalar_add(xb[:, dj, :], xb[:, dj, :], gvb[:, dj, :])

        # ---- attention ---------------------------------------------------
        with ExitStack() as bctx:
            psL = bctx.enter_context(tc.tile_pool(name="psL", bufs=2, space="PSUM"))
            psAV = bctx.enter_context(tc.tile_pool(name="psAV", bufs=4, space="PSUM"))
            psT = bctx.enter_context(tc.tile_pool(name="psT", bufs=2, space="PSUM"))
            attn = bctx.enter_context(tc.tile_pool(name="attn", bufs=6))
            eps = bctx.enter_context(tc.tile_pool(name="eps", bufs=4))

            for s in range(NSTRIPS):
                nsl = slice(s * 512, (s + 1) * 512)
                av = [psAV.tile([P, 512], F32, tag="av", name="av") for _ in range(4)]
                for mt in range(MT):
                    msl = slice(mt * P, (mt + 1) * P)
                    pl = psL.tile([P, 512], F32, tag="pl")
                    nc.tensor.matmul(
                        pl, k_sb[:, msl], q_sb[:, nsl],
                        start=True, stop=True,
                    )
                    at = attn.tile([P, 512], FR, tag="at")
                    nc.scalar.activation(
                        out=at, in_=pl,
                        func=mybir.ActivationFunctionType.Exp, scale=SCALE,
                    )
                    for j in range(4):
                        nc.tensor.matmul(
                            av[j][:, 0:258],
                            at[:, j * P:(j + 1) * P],
                            vT[:, mt, 0:258],
                            start=(mt == 0), stop=(mt == MT - 1),
                        )
                # strip epilogue
                pT = [psT.tile([P, 512], F32, tag="pT", name="pT") for _ in range(2)]
                for j in range(4):
                    rec = eps.tile([P, 1], F32, tag="rec")
                    nc.vector.reciprocal(rec, av[j][:, 256:257])
                    recg = eps.tile([P, 1], F32, tag="recg")
                    nc.vector.tensor_scalar_mul(recg, rec, gamma_bc)
                    o_sb = eps.tile([P, 256], F32, tag="o_sb")
                    nc.vector.tensor_scalar_mul(o_sb, av[j][:, 0:256], recg)
                    for dj in range(2):
                        nc.tensor.transpose(
                            pT[dj][:, j * P:(j + 1) * P],
                            o_sb[:, dj * P:(dj + 1) * P],
                            ident,
                        )
                for dj in range(2):
                    res = eps.tile([P, 512], F32, tag="res")
                    nc.vector.tensor_add(res, pT[dj], xb[:, dj, nsl])
                    nc.sync.dma_start(
                        out=o_dram[dj * P:(dj + 1) * P, nsl], in_=res
                    )


def _shard_inputs(x, context, Wq, bq, Wk, bk, Wv, bv, gamma):
    xb = np.ascontiguousarray(np.asarray(x, dtype=np.float32)).reshape(B, C, NPIX)
    cb = np.ascontiguousarray(np.asarray(context, dtype=np.float32)).reshape(B, C, NPIX)
    shared = {
        "WqT": np.ascontiguousarray(np.asarray(Wq, dtype=np.float32).T),
        "bq": np.ascontiguousarray(np.asarray(bq, dtype=np.float32)),
        "WkT": np.ascontiguousarray(np.asarray(Wk, dtype=np.float32).T),
        "bk": np.ascontiguousarray(np.asarray(bk, dtype=np.float32)),
        "WvT": np.ascontiguousarray(np.asarray(Wv, dtype=np.float32).T),
        "bv": np.ascontiguousarray(np.asarray(bv, dtype=np.float32)),
        "gamma": np.ascontiguousarray(np.asarray(gamma, dtype=np.float32)),
    }
    in_maps = []
    for core in range(N_CORES):
        b, half = core // 2, core % 2
        m = dict(shared)
        m["xs"] = np.ascontiguousarray(xb[b][:, half * NLOC:(half + 1) * NLOC])
        m["cf"] = np.ascontiguousarray(cb[b])
        in_maps.append(m)
    return in_maps


def _gather(results):
    out = np.empty((B, C, NPIX), dtype=np.float32)
    for core in range(N_CORES):
        b, half = core // 2, core % 2
        out[b][:, half * NLOC:(half + 1) * NLOC] = results[core]["out"]
    return out.reshape(B, C, H, W)


def run(inputs, trace=False, **kw):
    """Build (cached), run on the 8 NeuronCores, return (output, results)."""
    if "nc" not in _CACHE:
        _CACHE["nc"] = build_nc()
    nc = _CACHE["nc"]
    in_maps = _shard_inputs(**inputs)
    res = run_bass_kernel_spmd(
        nc, in_maps, core_ids=list(range(N_CORES)), trace=trace, **kw
    )
    return _gather(res.results), res


def kernel(**inputs) -> np.ndarray:
    out, _ = run(inputs, trace=False)
    return out


# revision 14
# speedup vs baseline: 489.7570x; 1.2711x over previous
"""Trainium2 Bass kernel for nn_CrossAttention (B=4, C=256, H=W=64, RC=32).

Sharding: 8 cores = (batch b in 0..3) x (query-pixel half in 0..1).
Each core gets x[b][:, nloc] (256 x 2048), the full context[b] (256 x 4096),
and replicated weights; it computes out[b][:, nloc] (256 x 2048).
No collectives: k/v are recomputed on both cores sharing a batch item
(cheap relative to the attention matmuls).

Per-core math (all on-chip, fp32 with float32r matmuls):
  q  (32,  nloc) = WqT.T @ x      (+bq)
  k  (32,  4096) = WkT.T @ ctx    (+bk)
  vT (4096, 257) = ctx.T @ WvT    (col 256 = ones -> free row-sum)
  per 512-col strip of n:
    for each of 32 m-tiles: logitsT (128m, 512n) = k_tile.T @ q_strip  (K=32)
      attnT = exp(logitsT * 1/sqrt(32))               [ScalarE]
      av[j] (128n, 257) += attnT[:, j*128:...].T @ vT_tile  (K=128)
    per 128-row n-tile j: normalize by col 256 (row-sum), scale by gamma,
      PE-transpose to (d, n), add x + gamma*bv, DMA out.
"""

import math

import numpy as np

import concourse.bass as bass
import concourse.mybir as mybir
import concourse.tile as tile
from concourse import bacc, masks
from concourse.bass_utils import run_bass_kernel_spmd

B, C, RC = 4, 256, 32
H = W = 64
NPIX = H * W          # 4096 query pixels per batch item
M = NPIX              # context pixels
NLOC = NPIX // 2      # query pixels per core
P = 128
N_CORES = 8
NSTRIPS = NLOC // 512  # 4
MT = M // P            # 32 m-tiles
F32 = mybir.dt.float32
FR = mybir.dt.float32r
BF = mybir.dt.bfloat16
SCALE = 1.0 / math.sqrt(RC)

_CACHE = {}


def _r(ap):
    """View an fp32 AP as float32r for single-pass PE matmuls."""
    return ap.bitcast(FR)


def _bcast_part(ap, p):
    """Partition-broadcast a DRAM AP of shape (k,) to (p, k) via step-0 AP."""
    return bass.AP(tensor=ap.tensor, offset=ap.offset, ap=[[0, p]] + list(ap.ap))


def build_nc(reps=1):
    nc = bacc.Bacc("TRN2", target_bir_lowering=False, debug=False)
    xs = nc.dram_tensor("xs", [C, NLOC], F32, kind="ExternalInput").ap()
    cfd = nc.dram_tensor("cf", [C, M], F32, kind="ExternalInput").ap()
    wqT = nc.dram_tensor("WqT", [C, RC], F32, kind="ExternalInput").ap()
    bq = nc.dram_tensor("bq", [RC], F32, kind="ExternalInput").ap()
    wkT = nc.dram_tensor("WkT", [C, RC], F32, kind="ExternalInput").ap()
    bk = nc.dram_tensor("bk", [RC], F32, kind="ExternalInput").ap()
    wvT = nc.dram_tensor("WvT", [C, C], F32, kind="ExternalInput").ap()
    bv = nc.dram_tensor("bv", [C], F32, kind="ExternalInput").ap()
    gamma = nc.dram_tensor("gamma", [1], F32, kind="ExternalInput").ap()
    o_dram = nc.dram_tensor("out", [C, NLOC], F32, kind="ExternalOutput").ap()

    with tile.TileContext(nc) as tc:
        for _ in range(reps):
            _emit(tc, xs, cfd, wqT, bq, wkT, bk, wvT, bv, gamma, o_dram)
    nc.compile()
    return nc


def _emit(tc, xs, cfd, wqT_d, bq, wkT_d, bk, wvT_d, bv, gamma, o_dram):
    nc = tc.nc
    from contextlib import ExitStack

    with ExitStack() as ctx:
        const = ctx.enter_context(tc.tile_pool(name="const", bufs=1))

        ident = const.tile([P, P], F32)
        masks.make_identity(nc, ident)

        # ---- loads, ordered by when PE needs them -----------------------
        # xf + wqT first (unblock q), then cf pieces + wkT/wvT (k, vT),
        # then biases; xb (residual copy) last. DMA issue is spread across
        # engine sequencers so SP doesn't serialize the prologue.
        xf = const.tile([P, 2, NLOC], FR)   # [p, ci, n] rounded, feeds q matmul
        for ci in range(2):
            nc.sync.dma_start(out=xf[:, ci, :], in_=_r(xs[ci * P:(ci + 1) * P, :]))
        wqT = const.tile([P, 2, RC], FR)    # [c_in_chunk, ci, r]
        wkT = const.tile([P, 2, RC], FR)
        wvT = const.tile([P, 2, C], FR)     # [c_in_chunk, ci, d]
        for ci in range(2):
            csl = slice(ci * P, (ci + 1) * P)
            nc.scalar.dma_start(out=wqT[:, ci, :], in_=_r(wqT_d[csl, :]))
            nc.scalar.dma_start(out=wkT[:, ci, :], in_=_r(wkT_d[csl, :]))
            nc.gpsimd.dma_start(out=wvT[:, ci, :], in_=_r(wvT_d[csl, :]))
        # context in 8 independently-landing pieces [ci][pc] of (128, 1024)
        cfp = [
            [
                const.tile([P, 1024], FR, name=f"cf_{ci}_{pc}", tag=f"cf_{ci}_{pc}")
                for pc in range(4)
            ]
            for ci in range(2)
        ]
        dma_engines = [nc.sync, nc.scalar, nc.gpsimd]
        di = 0
        for pc in range(4):
            for ci in range(2):
                eng = dma_engines[di % 3]
                di += 1
                eng.dma_start(
                    out=cfp[ci][pc],
                    in_=_r(cfd[ci * P:(ci + 1) * P, pc * 1024:(pc + 1) * 1024]),
                )
        bq_sb = const.tile([RC, 1], F32)
        nc.gpsimd.dma_start(out=bq_sb, in_=bq.unsqueeze(1))
        bk_sb = const.tile([RC, 1], F32)
        nc.gpsimd.dma_start(out=bk_sb, in_=bk.unsqueeze(1))
        bv_sb = const.tile([P, 2, 1], F32)  # [p, dj, 1]
        for dj in range(2):
            nc.gpsimd.dma_start(
                out=bv_sb[:, dj, :], in_=bv[dj * P:(dj + 1) * P].unsqueeze(1)
            )
        gamma_bc = const.tile([P, 1], F32)
        nc.gpsimd.dma_start(out=gamma_bc, in_=_bcast_part(gamma, P))
        xb = const.tile([P, 2, NLOC], F32)  # exact x copy for the residual
        for ci in range(2):
            nc.sync.dma_start(out=xb[:, ci, :], in_=xs[ci * P:(ci + 1) * P, :])

        # ---- projections -------------------------------------------------
        q_sb = const.tile([RC, NLOC], BF)
        k_sb = const.tile([RC, M], BF)
        # col 256 = 1.0 (free row-sum), col 257 = 0.0 (fp32r needs even N)
        vT = const.tile([P, MT, 264], BF)  # [m_in_tile, mt, d]
        for mt in range(MT):
            nc.gpsimd.memset(vT[:, mt, 256:258], 0.0)
            nc.gpsimd.memset(vT[:, mt, 256:257], 1.0)

        with tc.tile_pool(name="psQ", bufs=2, space="PSUM") as psQ, \
             tc.tile_pool(name="psV", bufs=2, space="PSUM") as psV:
            for sq in range(NSTRIPS):
                sl = slice(sq * 512, (sq + 1) * 512)
                pq = psQ.tile([RC, 512], F32, tag="pq")
                for ci in range(2):
                    nc.tensor.matmul(
                        pq, wqT[:, ci, :], xf[:, ci, sl],
                        start=(ci == 0), stop=(ci == 1),
                    )
                nc.vector.tensor_scalar_add(q_sb[:, sl], pq, bq_sb)
            # k and vT interleaved per cf piece, following DMA arrival order
            for pc in range(4):
                for sk in range(2):
                    sl = slice((pc * 2 + sk) * 512, (pc * 2 + sk + 1) * 512)
                    psl = slice(sk * 512, (sk + 1) * 512)
                    pk = psQ.tile([RC, 512], F32, tag="pq")
                    for ci in range(2):
                        nc.tensor.matmul(
                            pk, wkT[:, ci, :], cfp[ci][pc][:, psl],
                            start=(ci == 0), stop=(ci == 1),
                        )
                    nc.vector.tensor_scalar_add(k_sb[:, sl], pk, bk_sb)
                for mi in range(8):
                    mt = pc * 8 + mi
                    psl = slice(mi * P, (mi + 1) * P)
                    pv = psV.tile([P, C], F32, tag="pvv")
                    for ci in range(2):
                        nc.tensor.matmul(
                            pv, cfp[ci][pc][:, psl], wvT[:, ci, :],
                            start=(ci == 0), stop=(ci == 1),
                        )
                    nc.vector.tensor_copy(vT[:, mt, 0:256], pv)

        # xb = x + gamma*bv (per-partition), written in place over xf
        gvb = const.tile([P, 2, 1], F32)
        for dj in range(2):
            nc.vector.tensor_scalar_mul(gvb[:, dj, :], bv_sb[:, dj, :], gamma_bc)
            nc.vector.tensor_scalar_add(xb[:, dj, :], xb[:, dj, :], gvb[:, dj, :])

        # ---- attention ---------------------------------------------------
        with ExitStack() as bctx:
            psL = bctx.enter_context(tc.tile_pool(name="psL", bufs=2, space="PSUM"))
            psAV = bctx.enter_context(tc.tile_pool(name="psAV", bufs=4, space="PSUM"))
            psT = bctx.enter_context(tc.tile_pool(name="psT", bufs=2, space="PSUM"))
            attn = bctx.enter_context(tc.tile_pool(name="attn", bufs=6))
            eps = bctx.enter_context(tc.tile_pool(name="eps", bufs=4))

            for s in range(NSTRIPS):
                nsl = slice(s * 512, (s + 1) * 512)
                av = [psAV.tile([P, 512], F32, tag="av", name="av") for _ in range(4)]
                for mt in range(MT):
                    msl = slice(mt * P, (mt + 1) * P)
                    pl = psL.tile([P, 512], F32, tag="pl")
                    nc.tensor.matmul(
                        pl, k_sb[:, msl], q_sb[:, nsl],
                        start=True, stop=True,
                    )
                    at = attn.tile([P, 512], BF, tag="at")
                    nc.scalar.activation(
                        out=at, in_=pl,
                        func=mybir.ActivationFunctionType.Exp, scale=SCALE,
                    )
                    for j in range(4):
                        nc.tensor.matmul(
                            av[j][:, 0:258],
                            at[:, j * P:(j + 1) * P],
                            vT[:, mt, 0:258],
                            start=(mt == 0), stop=(mt == MT - 1),
                        )
                # strip epilogue
                pT = [psT.tile([P, 512], F32, tag="pT", name="pT") for _ in range(2)]
                for j in range(4):
                    rec = eps.tile([P, 1], F32, tag="rec")
                    nc.vector.reciprocal(rec, av[j][:, 256:257])
                    recg = eps.tile([P, 1], F32, tag="recg")
                    nc.vector.tensor_scalar_mul(recg, rec, gamma_bc)
                    o_sb = eps.tile([P, 256], F32, tag="o_sb")
                    nc.vector.tensor_scalar_mul(o_sb, av[j][:, 0:256], recg)
                    for dj in range(2):
                        nc.tensor.transpose(
                            pT[dj][:, j * P:(j + 1) * P],
                            o_sb[:, dj * P:(dj + 1) * P],
                            ident,
                        )
                for dj in range(2):
                    res = eps.tile([P, 512], F32, tag="res")
                    nc.vector.tensor_add(res, pT[dj], xb[:, dj, nsl])
                    nc.sync.dma_start(
                        out=o_dram[dj * P:(dj + 1) * P, nsl], in_=res
                    )


def _shard_inputs(x, context, Wq, bq, Wk, bk, Wv, bv, gamma):
    xb = np.ascontiguousarray(np.asarray(x, dtype=np.float32)).reshape(B, C, NPIX)
    cb = np.ascontiguousarray(np.asarray(context, dtype=np.float32)).reshape(B, C, NPIX)
    shared = {
        "WqT": np.ascontiguousarray(np.asarray(Wq, dtype=np.float32).T),
        "bq": np.ascontiguousarray(np.asarray(bq, dtype=np.float32)),
        "WkT": np.ascontiguousarray(np.asarray(Wk, dtype=np.float32).T),
        "bk": np.ascontiguousarray(np.asarray(bk, dtype=np.float32)),
        "WvT": np.ascontiguousarray(np.asarray(Wv, dtype=np.float32).T),
        "bv": np.ascontiguousarray(np.asarray(bv, dtype=np.float32)),
        "gamma": np.ascontiguousarray(np.asarray(gamma, dtype=np.float32)),
    }
    in_maps = []
    for core in range(N_CORES):
        b, half = core // 2, core % 2
        m = dict(shared)
        m["xs"] = np.ascontiguousarray(xb[b][:, half * NLOC:(half + 1) * NLOC])
        m["cf"] = np.ascontiguousarray(cb[b])
        in_maps.append(m)
    return in_maps


def _gather(results):
    out = np.empty((B, C, NPIX), dtype=np.float32)
    for core in range(N_CORES):
        b, half = core // 2, core % 2
        out[b][:, half * NLOC:(half + 1) * NLOC] = results[core]["out"]
    return out.reshape(B, C, H, W)


def run(inputs, trace=False, **kw):
    """Build (cached), run on the 8 NeuronCores, return (output, results)."""
    if "nc" not in _CACHE:
        _CACHE["nc"] = build_nc()
    nc = _CACHE["nc"]
    in_maps = _shard_inputs(**inputs)
    res = run_bass_kernel_spmd(
        nc, in_maps, core_ids=list(range(N_CORES)), trace=trace, **kw
    )
    return _gather(res.results), res


def kernel(**inputs) -> np.ndarray:
    out, _ = run(inputs, trace=False)
    return out
